# revision 6
# baseline (speedup 1.0000x reference)
"""Sparse-attention kernel for Trainium2 (Bass/Tile), data-parallel over batch.

Problem (hardcoded shapes):
  x [32, 256, 64, 64] f32, context [32, 256, 128] f32, W [256, 256] f32,
  mask [32, 128] bool.
  source = W @ context                          [B, 256, 128]
  attn   = softmax_L(x^T @ source + maskbias)   [B, 4096, 128]
  (reference's jnp.tile mask quirk => effective mask is mask[p % 32, l],
   independent of batch)
  out0   = source @ attn^T reshaped             [32, 256, 64, 64]
  out1   = attn^T reshaped                      [32, 128, 64, 64]

Strategy: 8 cores, 4 images each. Everything is computed in the transposed
attention layout attn_T [L=128 partitions, pos free], which makes every
matmul operand native-layout (zero transposes). Softmax over the partition
dim uses a fixed shift + log-sum-exp: column sums come from a ones-matmul
on the PE (which also broadcasts them to all 128 partitions), then
attn = exp(shifted - ln(colsum)).
"""

import numpy as np

import concourse.bass as bass
import concourse.tile as tile
from concourse import bacc, mybir
from concourse.bass_utils import run_bass_kernel_spmd

B, IDF, CDF, H, W_DIM, L = 32, 256, 256, 64, 64, 128
HXW = H * W_DIM
NCORES = 8
NB = B // NCORES  # images per core
SHIFT = 72.0  # exp1 shift; exact for any value via log-sum-exp
NEG = -1.0e30
PT = 512  # pos tile (one PSUM bank of fp32)
CHUNK = 2048  # pos chunk per DMA (8KB/partition lines)
F32 = mybir.dt.float32


def emit(tc, nc, x, ctxt, wt, mb, res, attn, repeats=1):
    with (
        tc.tile_pool(name="const", bufs=1) as const,
        tc.tile_pool(name="ctxp", bufs=4) as ctxp,
        tc.tile_pool(name="srcp", bufs=4) as srcp,
        tc.tile_pool(name="srctp", bufs=2) as srctp,
        tc.tile_pool(name="xp", bufs=8) as xp,
        tc.tile_pool(name="astage", bufs=2) as astage,
        tc.tile_pool(name="rstage", bufs=4) as rstage,
        tc.tile_pool(name="smp", bufs=3) as smp,
        tc.tile_pool(name="ps_a", bufs=3, space="PSUM") as ps_a,
        tc.tile_pool(name="ps_s", bufs=2, space="PSUM") as ps_s,
        tc.tile_pool(name="ps_r", bufs=3, space="PSUM") as ps_r,
    ):
        wt0 = const.tile([128, IDF], F32, tag="wt0")
        wt1 = const.tile([128, IDF], F32, tag="wt1")
        nc.sync.dma_start(wt0[:], wt[0:128, :])
        nc.sync.dma_start(wt1[:], wt[128:256, :])
        ones = const.tile([128, 128], F32, tag="ones")
        nc.vector.memset(ones[:], 1.0)
        mbias = const.tile([128, PT], F32, tag="mbias")
        nc.sync.dma_start(mbias[:], mb[:, :])

        for b_rep in range(NB * repeats):
            b = b_rep % NB
            ctx0 = ctxp.tile([128, L], F32, tag="ctx")
            ctx1 = ctxp.tile([128, L], F32, tag="ctx")
            nc.sync.dma_start(ctx0[:], ctxt[b, 0:128, :])
            nc.sync.dma_start(ctx1[:], ctxt[b, 128:256, :])

            # srcT [L, IDF] = ctx^T @ W^T = (W @ ctx)^T
            ps_t = ps_a.tile([128, IDF], F32, tag="psa")
            nc.tensor.matmul(ps_t[:], ctx0[:], wt0[:], start=True, stop=False)
            nc.tensor.matmul(ps_t[:], ctx1[:], wt1[:], start=False, stop=True)
            srcT = srctp.tile([128, IDF], F32, tag="srcT")
            nc.any.tensor_copy(srcT[:], ps_t[:])

            # src tiles [idf_tile=128, L]
            srcs = []
            for it in range(2):
                isl = bass.ts(it, 128)
                ps_src = ps_a.tile([128, L], F32, tag="psa")
                nc.tensor.matmul(
                    ps_src[:], wt0[:, isl], ctx0[:], start=True, stop=False
                )
                nc.tensor.matmul(
                    ps_src[:], wt1[:, isl], ctx1[:], start=False, stop=True
                )
                s = srcp.tile([128, L], F32, tag="src")
                nc.any.tensor_copy(s[:], ps_src[:])
                srcs.append(s)

            for c in range(HXW // CHUNK):
                csl = bass.ts(c, CHUNK)
                x0 = xp.tile([128, CHUNK], F32, tag="xc")
                x1 = xp.tile([128, CHUNK], F32, tag="xc")
                nc.sync.dma_start(x0[:], x[b, 0:128, csl])
                nc.sync.dma_start(x1[:], x[b, 128:256, csl])
                a_st = astage.tile([128, CHUNK], F32, tag="ast")
                r_st0 = rstage.tile([128, CHUNK], F32, tag="rst")
                r_st1 = rstage.tile([128, CHUNK], F32, tag="rst")

                for j in range(CHUNK // PT):
                    sl = bass.ts(j, PT)
                    psA = ps_a.tile([128, PT], F32, tag="psa")
                    nc.tensor.matmul(
                        psA[:], srcs[0][:], x0[:, sl], start=True, stop=False
                    )
                    nc.tensor.matmul(
                        psA[:], srcs[1][:], x1[:, sl], start=False, stop=True
                    )
                    shifted = smp.tile([128, PT], F32, tag="shifted")
                    nc.vector.tensor_add(shifted[:], psA[:], mbias[:])
                    e1 = smp.tile([128, PT], F32, tag="e1")
                    nc.scalar.activation(
                        e1[:], shifted[:], mybir.ActivationFunctionType.Exp
                    )
                    psS = ps_s.tile([128, PT], F32, tag="pss")
                    nc.tensor.matmul(psS[:], ones[:], e1[:], start=True, stop=True)
                    scr = smp.tile([128, PT], F32, tag="scr")
                    rec = smp.tile([128, PT], F32, tag="rec")
                    nc.vector.reciprocal_approx_accurate(
                        out=rec[:], in_=psS[:], scratch=scr[:]
                    )
                    nc.vector.tensor_mul(a_st[:, sl], e1[:], rec[:])
                    for it, r_st in ((0, r_st0), (1, r_st1)):
                        psR = ps_r.tile([128, PT], F32, tag="psr")
                        nc.tensor.matmul(
                            psR[:],
                            srcT[:, bass.ts(it, 128)],
                            a_st[:, sl],
                            start=True,
                            stop=True,
                        )
                        nc.any.tensor_copy(r_st[:, sl], psR[:])

                nc.sync.dma_start(attn[b, :, csl], a_st[:])
                nc.sync.dma_start(res[b, 0:128, csl], r_st0[:])
                nc.sync.dma_start(res[b, 128:256, csl], r_st1[:])


def build_nc(repeats=1):
    nc = bacc.Bacc(
        "TRN2",
        target_bir_lowering=False,
        debug=False,
        enable_asserts=False,
        num_devices=NCORES,
    )
    x = nc.dram_tensor("x", [NB, IDF, HXW], F32, kind="ExternalInput").ap()
    ctxt = nc.dram_tensor("ctxt", [NB, CDF, L], F32, kind="ExternalInput").ap()
    wt = nc.dram_tensor("wt", [CDF, IDF], F32, kind="ExternalInput").ap()
    mb = nc.dram_tensor("mbias", [L, PT], F32, kind="ExternalInput").ap()
    res = nc.dram_tensor("res", [NB, IDF, HXW], F32, kind="ExternalOutput").ap()
    attn = nc.dram_tensor("attn", [NB, L, HXW], F32, kind="ExternalOutput").ap()
    with tile.TileContext(nc) as tc:
        emit(tc, nc, x, ctxt, wt, mb, res, attn, repeats=repeats)
    nc.compile()
    return nc


def make_in_maps(x, context, W, mask):
    x = np.ascontiguousarray(np.asarray(x, dtype=np.float32)).reshape(B, IDF, HXW)
    context = np.ascontiguousarray(np.asarray(context, dtype=np.float32))
    wt = np.ascontiguousarray(np.asarray(W, dtype=np.float32).T)
    mask = np.asarray(mask)
    # effective mask for (p, l) is mask[p % 32, l]; build [L, PT] additive bias
    mb_small = np.where(mask, np.float32(NEG), np.float32(0.0)).astype(np.float32)
    mb_t = np.tile(mb_small.T, (1, PT // mask.shape[0]))  # [L, PT]
    mb_t = (mb_t - np.float32(SHIFT)).astype(np.float32)
    in_maps = []
    for r in range(NCORES):
        sl = slice(r * NB, (r + 1) * NB)
        in_maps.append(
            {
                "x": np.ascontiguousarray(x[sl]),
                "ctxt": np.ascontiguousarray(context[sl]),
                "wt": wt,
                "mbias": mb_t,
            }
        )
    return in_maps


_NC_CACHE = None


def kernel(x, context, W, mask):
    global _NC_CACHE
    if _NC_CACHE is None:
        _NC_CACHE = build_nc()
    nc = _NC_CACHE
    in_maps = make_in_maps(x, context, W, mask)
    out = run_bass_kernel_spmd(nc, in_maps, core_ids=list(range(NCORES)))
    res = np.concatenate([m["res"] for m in out.results], axis=0)
    attn = np.concatenate([m["attn"] for m in out.results], axis=0)
    return res.reshape(B, IDF, H, W_DIM), attn.reshape(B, L, H, W_DIM)


# revision 17
# speedup vs baseline: 1.2092x; 1.2092x over previous
"""Sparse-attention kernel for Trainium2 (Bass/Tile), data-parallel over batch.

Problem (hardcoded shapes):
  x [32, 256, 64, 64] f32, context [32, 256, 128] f32, W [256, 256] f32,
  mask [32, 128] bool.
  source = W @ context                          [B, 256, 128]
  attn   = softmax_L(x^T @ source + maskbias)   [B, 4096, 128]
  (reference's jnp.tile mask quirk => effective mask is mask[p % 32, l],
   independent of batch)
  out0   = source @ attn^T reshaped             [32, 256, 64, 64]
  out1   = attn^T reshaped                      [32, 128, 64, 64]

Strategy: 8 cores, 4 images each. Everything is computed in the transposed
attention layout attn_T [L=128 partitions, pos free], which makes every
matmul operand native-layout (zero transposes). Softmax over the partition
dim uses a fixed shift + log-sum-exp: column sums come from a ones-matmul
on the PE (which also broadcasts them to all 128 partitions), then
attn = exp(shifted - ln(colsum)).
"""

import numpy as np

import concourse.bass as bass
import concourse.tile as tile
from concourse import bacc, mybir
from concourse.bass_utils import run_bass_kernel_spmd

B, IDF, CDF, H, W_DIM, L = 32, 256, 256, 64, 64, 128
HXW = H * W_DIM
NCORES = 8
NB = B // NCORES  # images per core
SHIFT = 72.0  # exp1 shift; exact for any value via log-sum-exp
NEG = -1.0e30
PT = 512  # pos tile (one PSUM bank of fp32)
CHUNK = 2048  # pos chunk per DMA (8KB/partition lines)
F32 = mybir.dt.float32
F32R = mybir.dt.float32r

# fp32r runs the PE at 4x fp32 speed (1 cycle/col at N>=256); precision is
# hardware-reduced vs fp32 — each site toggled independently, validated on HW.
F32R_LOGITS = False
F32R_SUM = True
F32R_PV = True


def _mm(nc, out, lhsT, rhs, start, stop, fast):
    nc.tensor.matmul(out, lhsT, rhs, start=start, stop=stop)


def emit(tc, nc, x, ctxt, wt, mb, res, attn, repeats=1):
    with (
        tc.tile_pool(name="const", bufs=1) as const,
        tc.tile_pool(name="ctxp", bufs=4) as ctxp,
        tc.tile_pool(name="srcp", bufs=4) as srcp,
        tc.tile_pool(name="srctp", bufs=2) as srctp,
        tc.tile_pool(name="xp", bufs=8) as xp,
        tc.tile_pool(name="astage", bufs=2) as astage,
        tc.tile_pool(name="rstage", bufs=4) as rstage,
        tc.tile_pool(name="smp", bufs=3) as smp,
        tc.tile_pool(name="ps_a", bufs=3, space="PSUM") as ps_a,
        tc.tile_pool(name="ps_s", bufs=2, space="PSUM") as ps_s,
        tc.tile_pool(name="ps_r", bufs=3, space="PSUM") as ps_r,
    ):
        wt0 = const.tile([128, IDF], F32, tag="wt0")
        wt1 = const.tile([128, IDF], F32, tag="wt1")
        nc.sync.dma_start(wt0[:], wt[0:128, :])
        nc.sync.dma_start(wt1[:], wt[128:256, :])
        ones = const.tile([128, 128], F32R if F32R_SUM else F32, tag="ones")
        if F32R_SUM:
            ones_f = const.tile([128, 128], F32, tag="ones_f")
            nc.vector.memset(ones_f[:], 1.0)
            nc.any.tensor_copy(ones[:], ones_f[:])
        else:
            nc.vector.memset(ones[:], 1.0)
        mbias = const.tile([128, PT], F32, tag="mbias")
        nc.sync.dma_start(mbias[:], mb[:, :])

        for b_rep in range(NB * repeats):
            b = b_rep % NB
            ctx0 = ctxp.tile([128, L], F32, tag="ctx")
            ctx1 = ctxp.tile([128, L], F32, tag="ctx")
            nc.sync.dma_start(ctx0[:], ctxt[b, 0:128, :])
            nc.sync.dma_start(ctx1[:], ctxt[b, 128:256, :])

            # srcT [L, IDF] = ctx^T @ W^T = (W @ ctx)^T
            ps_t = ps_a.tile([128, IDF], F32, tag="psa")
            nc.tensor.matmul(ps_t[:], ctx0[:], wt0[:], start=True, stop=False)
            nc.tensor.matmul(ps_t[:], ctx1[:], wt1[:], start=False, stop=True)
            srcT = srctp.tile([128, IDF], F32R if F32R_PV else F32, tag="srcT")
            nc.any.tensor_copy(srcT[:], ps_t[:])

            # src tiles [idf_tile=128, L]
            srcs = []
            for it in range(2):
                isl = bass.ts(it, 128)
                ps_src = ps_a.tile([128, L], F32, tag="psa")
                nc.tensor.matmul(
                    ps_src[:], wt0[:, isl], ctx0[:], start=True, stop=False
                )
                nc.tensor.matmul(
                    ps_src[:], wt1[:, isl], ctx1[:], start=False, stop=True
                )
                s = srcp.tile([128, L], F32R if F32R_LOGITS else F32, tag="src")
                nc.any.tensor_copy(s[:], ps_src[:])
                srcs.append(s)

            for c in range(HXW // CHUNK):
                csl = bass.ts(c, CHUNK)
                x0 = xp.tile([128, CHUNK], F32R if F32R_LOGITS else F32, tag="xc")
                x1 = xp.tile([128, CHUNK], F32R if F32R_LOGITS else F32, tag="xc")
                nc.sync.dma_start(x0[:], x[b, 0:128, csl])
                nc.sync.dma_start(x1[:], x[b, 128:256, csl])
                a_st = astage.tile([128, CHUNK], F32R if F32R_PV else F32, tag="ast")
                r_st0 = rstage.tile([128, CHUNK], F32, tag="rst")
                r_st1 = rstage.tile([128, CHUNK], F32, tag="rst")

                for j in range(CHUNK // PT):
                    sl = bass.ts(j, PT)
                    psA = ps_a.tile([128, PT], F32, tag="psa")
                    _mm(nc, psA[:], srcs[0][:], x0[:, sl], True, False, F32R_LOGITS)
                    _mm(nc, psA[:], srcs[1][:], x1[:, sl], False, True, F32R_LOGITS)
                    shifted = smp.tile([128, PT], F32, tag="shifted")
                    nc.vector.tensor_add(shifted[:], psA[:], mbias[:])
                    e1 = smp.tile([128, PT], F32R if F32R_SUM else F32, tag="e1")
                    nc.scalar.activation(
                        e1[:], shifted[:], mybir.ActivationFunctionType.Exp
                    )
                    psS = ps_s.tile([128, PT], F32, tag="pss")
                    _mm(nc, psS[:], ones[:], e1[:], True, True, F32R_SUM)
                    scr = smp.tile([128, PT], F32, tag="scr")
                    rec = smp.tile([128, PT], F32, tag="rec")
                    nc.vector.reciprocal_approx_accurate(
                        out=rec[:], in_=psS[:], scratch=scr[:]
                    )
                    nc.vector.tensor_mul(a_st[:, sl], e1[:], rec[:])
                    for it, r_st in ((0, r_st0), (1, r_st1)):
                        psR = ps_r.tile([128, PT], F32, tag="psr")
                        _mm(
                            nc,
                            psR[:],
                            srcT[:, bass.ts(it, 128)],
                            a_st[:, sl],
                            True,
                            True,
                            F32R_PV,
                        )
                        nc.any.tensor_copy(r_st[:, sl], psR[:])

                nc.sync.dma_start(attn[b, :, csl], a_st[:])
                nc.sync.dma_start(res[b, 0:128, csl], r_st0[:])
                nc.sync.dma_start(res[b, 128:256, csl], r_st1[:])


def build_nc(repeats=1):
    nc = bacc.Bacc(
        "TRN2",
        target_bir_lowering=False,
        debug=False,
        enable_asserts=False,
        num_devices=NCORES,
    )
    x = nc.dram_tensor(
        "x", [NB, IDF, HXW], F32R if F32R_LOGITS else F32, kind="ExternalInput"
    ).ap()
    ctxt = nc.dram_tensor("ctxt", [NB, CDF, L], F32, kind="ExternalInput").ap()
    wt = nc.dram_tensor("wt", [CDF, IDF], F32, kind="ExternalInput").ap()
    mb = nc.dram_tensor("mbias", [L, PT], F32, kind="ExternalInput").ap()
    res = nc.dram_tensor("res", [NB, IDF, HXW], F32, kind="ExternalOutput").ap()
    attn = nc.dram_tensor(
        "attn", [NB, L, HXW], F32R if F32R_PV else F32, kind="ExternalOutput"
    ).ap()
    with tile.TileContext(nc) as tc:
        emit(tc, nc, x, ctxt, wt, mb, res, attn, repeats=repeats)
    nc.compile()
    return nc


def make_in_maps(x, context, W, mask):
    x = np.ascontiguousarray(np.asarray(x, dtype=np.float32)).reshape(B, IDF, HXW)
    context = np.ascontiguousarray(np.asarray(context, dtype=np.float32))
    wt = np.ascontiguousarray(np.asarray(W, dtype=np.float32).T)
    mask = np.asarray(mask)
    # effective mask for (p, l) is mask[p % 32, l]; build [L, PT] additive bias
    mb_small = np.where(mask, np.float32(NEG), np.float32(0.0)).astype(np.float32)
    mb_t = np.tile(mb_small.T, (1, PT // mask.shape[0]))  # [L, PT]
    mb_t = (mb_t - np.float32(SHIFT)).astype(np.float32)
    in_maps = []
    for r in range(NCORES):
        sl = slice(r * NB, (r + 1) * NB)
        in_maps.append(
            {
                "x": np.ascontiguousarray(x[sl]),
                "ctxt": np.ascontiguousarray(context[sl]),
                "wt": wt,
                "mbias": mb_t,
            }
        )
    return in_maps


_NC_CACHE = None


def kernel(x, context, W, mask):
    global _NC_CACHE
    if _NC_CACHE is None:
        _NC_CACHE = build_nc()
    nc = _NC_CACHE
    in_maps = make_in_maps(x, context, W, mask)
    out = run_bass_kernel_spmd(nc, in_maps, core_ids=list(range(NCORES)))
    res = np.concatenate([m["res"] for m in out.results], axis=0)
    attn = np.concatenate([m["attn"] for m in out.results], axis=0)
    return res.reshape(B, IDF, H, W_DIM), attn.reshape(B, L, H, W_DIM)


# revision 23
# speedup vs baseline: 1.4505x; 1.1995x over previous
"""Sparse-attention kernel for Trainium2 (Bass/Tile), data-parallel over batch.

Problem (hardcoded shapes):
  x [32, 256, 64, 64] f32, context [32, 256, 128] f32, W [256, 256] f32,
  mask [32, 128] bool.
  source = W @ context                          [B, 256, 128]
  attn   = softmax_L(x^T @ source + maskbias)   [B, 4096, 128]
  (reference's jnp.tile mask quirk => effective mask is mask[p % 32, l],
   independent of batch)
  out0   = source @ attn^T reshaped             [32, 256, 64, 64]
  out1   = attn^T reshaped                      [32, 128, 64, 64]

Strategy: 8 cores, 4 images each. Everything is computed in the transposed
attention layout attn_T [L=128 partitions, pos free], which makes every
matmul operand native-layout (zero transposes). Softmax over the partition
dim uses a fixed shift + log-sum-exp: column sums come from a ones-matmul
on the PE (which also broadcasts them to all 128 partitions), then
attn = exp(shifted - ln(colsum)).
"""

import numpy as np

import concourse.bass as bass
import concourse.tile as tile
from concourse import bacc, mybir
from concourse.bass_utils import run_bass_kernel_spmd

B, IDF, CDF, H, W_DIM, L = 32, 256, 256, 64, 64, 128
HXW = H * W_DIM
NCORES = 8
NB = B // NCORES  # images per core
SHIFT = 72.0  # exp1 shift; exact for any value via log-sum-exp
NEG = -1.0e30
PT = 512  # pos tile (one PSUM bank of fp32)
CHUNK = 2048  # pos chunk per DMA (8KB/partition lines)
F32 = mybir.dt.float32
F32R = mybir.dt.float32r

# fp32r runs the PE at 4x fp32 speed (1 cycle/col at N>=256); precision is
# hardware-reduced vs fp32 — each site toggled independently, validated on HW.
F32R_LOGITS = False
F32R_SUM = True
F32R_PV = True


def _mm(nc, out, lhsT, rhs, start, stop, fast):
    nc.tensor.matmul(out, lhsT, rhs, start=start, stop=stop)


def emit(tc, nc, x, ctxt, wt, mb, res, attn, repeats=1):
    with (
        tc.tile_pool(name="const", bufs=1) as const,
        tc.tile_pool(name="ctxp", bufs=4) as ctxp,
        tc.tile_pool(name="srcp", bufs=4) as srcp,
        tc.tile_pool(name="srctp", bufs=2) as srctp,
        tc.tile_pool(name="xp", bufs=10) as xp,
        tc.tile_pool(name="astage", bufs=2) as astage,
        tc.tile_pool(name="rstage", bufs=4) as rstage,
        tc.tile_pool(name="smp", bufs=3) as smp,
        tc.tile_pool(name="ps_a", bufs=3, space="PSUM") as ps_a,
        tc.tile_pool(name="ps_s", bufs=2, space="PSUM") as ps_s,
        tc.tile_pool(name="ps_r", bufs=3, space="PSUM") as ps_r,
    ):
        wt0 = const.tile([128, IDF], F32, tag="wt0")
        wt1 = const.tile([128, IDF], F32, tag="wt1")
        nc.sync.dma_start(wt0[:], wt[0:128, :])
        nc.sync.dma_start(wt1[:], wt[128:256, :])
        ones = const.tile([128, 128], F32R if F32R_SUM else F32, tag="ones")
        if F32R_SUM:
            ones_f = const.tile([128, 128], F32, tag="ones_f")
            nc.vector.memset(ones_f[:], 1.0)
            nc.any.tensor_copy(ones[:], ones_f[:])
        else:
            nc.vector.memset(ones[:], 1.0)
        mbias = const.tile([128, PT], F32, tag="mbias")
        nc.sync.dma_start(mbias[:], mb[:, :])

        for b_rep in range(NB * repeats):
            b = b_rep % NB
            ctx0 = ctxp.tile([128, L], F32, tag="ctx")
            ctx1 = ctxp.tile([128, L], F32, tag="ctx")
            nc.sync.dma_start(ctx0[:], ctxt[b, 0:128, :])
            nc.sync.dma_start(ctx1[:], ctxt[b, 128:256, :])

            # srcT [L, IDF] = ctx^T @ W^T = (W @ ctx)^T
            ps_t = ps_a.tile([128, IDF], F32, tag="psa")
            nc.tensor.matmul(ps_t[:], ctx0[:], wt0[:], start=True, stop=False)
            nc.tensor.matmul(ps_t[:], ctx1[:], wt1[:], start=False, stop=True)
            srcT = srctp.tile([128, IDF], F32R if F32R_PV else F32, tag="srcT")
            nc.any.tensor_copy(srcT[:], ps_t[:])

            # src tiles [idf_tile=128, L]
            srcs = []
            for it in range(2):
                isl = bass.ts(it, 128)
                ps_src = ps_a.tile([128, L], F32, tag="psa")
                nc.tensor.matmul(
                    ps_src[:], wt0[:, isl], ctx0[:], start=True, stop=False
                )
                nc.tensor.matmul(
                    ps_src[:], wt1[:, isl], ctx1[:], start=False, stop=True
                )
                s = srcp.tile([128, L], F32R if F32R_LOGITS else F32, tag="src")
                nc.any.tensor_copy(s[:], ps_src[:])
                srcs.append(s)

            for c in range(HXW // CHUNK):
                csl = bass.ts(c, CHUNK)
                x0 = xp.tile([128, CHUNK], F32R if F32R_LOGITS else F32, tag="xc")
                x1 = xp.tile([128, CHUNK], F32R if F32R_LOGITS else F32, tag="xc")
                nc.sync.dma_start(x0[:], x[b, 0:128, csl])
                nc.sync.dma_start(x1[:], x[b, 128:256, csl])
                a_st = astage.tile([128, CHUNK], F32R if F32R_PV else F32, tag="ast")
                r_st0 = rstage.tile([128, CHUNK], F32, tag="rst")
                r_st1 = rstage.tile([128, CHUNK], F32, tag="rst")

                for j in range(CHUNK // PT):
                    sl = bass.ts(j, PT)
                    gsl = bass.ds(c * CHUNK + j * PT, PT)
                    psA = ps_a.tile([128, PT], F32, tag="psa")
                    _mm(nc, psA[:], srcs[0][:], x0[:, sl], True, False, F32R_LOGITS)
                    _mm(nc, psA[:], srcs[1][:], x1[:, sl], False, True, F32R_LOGITS)
                    shifted = smp.tile([128, PT], F32, tag="shifted")
                    nc.vector.tensor_add(shifted[:], psA[:], mbias[:])
                    e1 = smp.tile([128, PT], F32R if F32R_SUM else F32, tag="e1")
                    nc.scalar.activation(
                        e1[:], shifted[:], mybir.ActivationFunctionType.Exp
                    )
                    psS = ps_s.tile([128, PT], F32, tag="pss")
                    _mm(nc, psS[:], ones[:], e1[:], True, True, F32R_SUM)
                    scr = smp.tile([128, PT], F32, tag="scr")
                    rec = smp.tile([128, PT], F32, tag="rec")
                    nc.vector.reciprocal_approx_accurate(
                        out=rec[:], in_=psS[:], scratch=scr[:]
                    )
                    nc.vector.tensor_mul(a_st[:, sl], e1[:], rec[:])
                    nc.sync.dma_start(attn[b, :, gsl], a_st[:, sl])
                    for it, r_st in ((0, r_st0), (1, r_st1)):
                        psR = ps_r.tile([128, PT], F32, tag="psr")
                        _mm(
                            nc,
                            psR[:],
                            srcT[:, bass.ts(it, 128)],
                            a_st[:, sl],
                            True,
                            True,
                            F32R_PV,
                        )
                        nc.any.tensor_copy(r_st[:, sl], psR[:])
                        nc.sync.dma_start(
                            res[b, it * 128 : (it + 1) * 128, gsl], r_st[:, sl]
                        )


def build_nc(repeats=1):
    nc = bacc.Bacc(
        "TRN2",
        target_bir_lowering=False,
        debug=False,
        enable_asserts=False,
        num_devices=NCORES,
    )
    x = nc.dram_tensor(
        "x", [NB, IDF, HXW], F32R if F32R_LOGITS else F32, kind="ExternalInput"
    ).ap()
    ctxt = nc.dram_tensor("ctxt", [NB, CDF, L], F32, kind="ExternalInput").ap()
    wt = nc.dram_tensor("wt", [CDF, IDF], F32, kind="ExternalInput").ap()
    mb = nc.dram_tensor("mbias", [L, PT], F32, kind="ExternalInput").ap()
    res = nc.dram_tensor("res", [NB, IDF, HXW], F32, kind="ExternalOutput").ap()
    attn = nc.dram_tensor(
        "attn", [NB, L, HXW], F32R if F32R_PV else F32, kind="ExternalOutput"
    ).ap()
    with tile.TileContext(nc) as tc:
        emit(tc, nc, x, ctxt, wt, mb, res, attn, repeats=repeats)
    nc.compile()
    return nc


def make_in_maps(x, context, W, mask):
    x = np.ascontiguousarray(np.asarray(x, dtype=np.float32)).reshape(B, IDF, HXW)
    context = np.ascontiguousarray(np.asarray(context, dtype=np.float32))
    wt = np.ascontiguousarray(np.asarray(W, dtype=np.float32).T)
    mask = np.asarray(mask)
    # effective mask for (p, l) is mask[p % 32, l]; build [L, PT] additive bias
    mb_small = np.where(mask, np.float32(NEG), np.float32(0.0)).astype(np.float32)
    mb_t = np.tile(mb_small.T, (1, PT // mask.shape[0]))  # [L, PT]
    mb_t = (mb_t - np.float32(SHIFT)).astype(np.float32)
    in_maps = []
    for r in range(NCORES):
        sl = slice(r * NB, (r + 1) * NB)
        in_maps.append(
            {
                "x": np.ascontiguousarray(x[sl]),
                "ctxt": np.ascontiguousarray(context[sl]),
                "wt": wt,
                "mbias": mb_t,
            }
        )
    return in_maps


_NC_CACHE = None


def kernel(x, context, W, mask):
    global _NC_CACHE
    if _NC_CACHE is None:
        _NC_CACHE = build_nc()
    nc = _NC_CACHE
    in_maps = make_in_maps(x, context, W, mask)
    out = run_bass_kernel_spmd(nc, in_maps, core_ids=list(range(NCORES)))
    res = np.concatenate([m["res"] for m in out.results], axis=0)
    attn = np.concatenate([m["attn"] for m in out.results], axis=0)
    return res.reshape(B, IDF, H, W_DIM), attn.reshape(B, L, H, W_DIM)
